# revision 1
# baseline (speedup 1.0000x reference)
"""Multi-head attention (B=2, S=2048, D=1024, H=16) on 8 Trainium2 cores.

Sharding: core c handles batch b=c//4 and head group g=c%4 (4 heads, 256 dims).
Per-core math (all matmuls fp32r on the PE):
  Q^T = (Wq/8).T @ xq^T          [256, 2048]   (scale folded into Wq)
  K^T = Wk.T @ xk^T              [256, 2048]
  V   = xv @ Wv                  [2048, 256]   (xv rows >= valid_len zeroed on host)
  per head h, transposed scores S^T = K_h @ Q_h^T  [2048k, 2048q]
  p^T = exp(S^T)                 (no max subtraction; scores are O(10))
  [O_h^T; denom] = [V_h | mask].T @ p^T   (mask column gives the masked softmax
                                           denominator for free)
  O_h^T normalized by 1/denom (gpsimd partition_broadcast of reciprocal)
  out_partial = O @ Wo_g         [2048, 1024]
Host sums the 4 head-group partials per batch.
"""

import numpy as np
from contextlib import ExitStack

B, S, D, H = 2, 2048, 1024, 16
HG = 4  # heads per core
DH = 64
GP = HG * DH  # 256: per-core projection width
P = 128
QB = 512  # q block (matmul free dim)
NQB = S // QB
NKT = S // P  # 16 k tiles
NKC = D // P  # 8 contraction chunks over D
NST = S // P  # 16 seq tiles for output

_BUILD_CACHE = {}


def _build(reps=1, loop_n=1):
    key = (reps, loop_n)
    if key in _BUILD_CACHE:
        return _BUILD_CACHE[key]
    import concourse.bass as bass
    import concourse.tile as tile
    from concourse import bacc, mybir

    f32 = mybir.dt.float32
    f32r = mybir.dt.float32r

    nc = bacc.Bacc("TRN2", target_bir_lowering=False, debug=False, num_devices=8)

    xq = nc.dram_tensor("xq_t", [D, S], f32r, kind="ExternalInput").ap()
    xk = nc.dram_tensor("xk_t", [D, S], f32r, kind="ExternalInput").ap()
    xv = nc.dram_tensor("xv_t", [D, S], f32r, kind="ExternalInput").ap()
    wq = nc.dram_tensor("wq", [D, GP], f32r, kind="ExternalInput").ap()
    wk = nc.dram_tensor("wk", [D, GP], f32r, kind="ExternalInput").ap()
    wv = nc.dram_tensor("wv", [D, GP], f32r, kind="ExternalInput").ap()
    wo = nc.dram_tensor("wo", [GP, D], f32r, kind="ExternalInput").ap()
    maskd = nc.dram_tensor("mask_t", [P, NKT], f32, kind="ExternalInput").ap()
    outp = nc.dram_tensor("out_p", [S, D], f32, kind="ExternalOutput").ap()

    with tile.TileContext(nc) as tc:
        with ExitStack() as ctx:
            wpool = ctx.enter_context(tc.tile_pool(name="weights", bufs=1))
            xpool = ctx.enter_context(tc.tile_pool(name="xstream", bufs=3))
            qkpool = ctx.enter_context(tc.tile_pool(name="qk", bufs=1))
            vpool = ctx.enter_context(tc.tile_pool(name="v", bufs=1))
            otpool = ctx.enter_context(tc.tile_pool(name="ot", bufs=1))
            ppool = ctx.enter_context(tc.tile_pool(name="p", bufs=4))
            rcpool = ctx.enter_context(tc.tile_pool(name="rc", bufs=4))
            bcpool = ctx.enter_context(tc.tile_pool(name="bc", bufs=4))
            opool = ctx.enter_context(tc.tile_pool(name="oev", bufs=4))
            # PSUM: 4 one-bank slots (2 held by the live attention pair,
            # 2 rotating for projections / output) + 2 two-bank score slots.
            ps_a = ctx.enter_context(tc.tile_pool(name="ps_a", bufs=4, space="PSUM"))
            ps_s = ctx.enter_context(tc.tile_pool(name="ps_s", bufs=2, space="PSUM"))

            # ---- constants / weights ----
            wq_s = wpool.tile([P, NKC, GP], f32r, tag="wq")
            wk_s = wpool.tile([P, NKC, GP], f32r, tag="wk")
            wv_s = wpool.tile([P, NKC, GP], f32r, tag="wv")
            wo_s = wpool.tile([P, GP // P, D], f32r, tag="wo")
            mask_s = wpool.tile([P, NKT], f32, tag="mask")
            nc.sync.dma_start(wk_s[:], wk.rearrange("(c p) m -> p c m", p=P))

            xq_r = xq.rearrange("(c p) s -> p c s", p=P)
            xk_r = xk.rearrange("(c p) s -> p c s", p=P)
            xv_r = xv.rearrange("(c p) s -> p c s", p=P)

            from contextlib import nullcontext
            for _ in range(reps):
              with (tc.For_i(0, loop_n, 1) if loop_n > 1 else nullcontext()):
                  # ---- residents ----
                  QT = [qkpool.tile([P, S], f32r, tag=f"qt{t}", name=f"qt{t}") for t in range(2)]
                  KT = [qkpool.tile([P, S], f32r, tag=f"kt{t}", name=f"kt{t}") for t in range(2)]
                  V_sb = vpool.tile([P, NKT, HG, DH + 1], f32r, tag="vsb")
                  OT = [otpool.tile([P, S], f32r, tag=f"ot{t}", name=f"ot{t}") for t in range(2)]

                  fillers = []  # small PE work units woven into the kt loops

                  def qk_proj_dma(x_r, qb):
                      xt = xpool.tile([P, NKC, QB], f32r, tag="xs", name="xt")
                      nc.sync.dma_start(xt[:], x_r[:, :, qb * QB : (qb + 1) * QB])
                      return xt

                  def qk_proj_mt(w_t, dst, qb, xt, mt):
                      ps = ps_a.tile([P, QB], f32, tag="acc", name="ps")
                      for c in range(NKC):
                          nc.tensor.matmul(
                              ps[:],
                              lhsT=w_t[:, c, mt * P : (mt + 1) * P],
                              rhs=xt[:, c, :],
                              start=(c == 0),
                              stop=(c == NKC - 1),
                          )
                      nc.vector.tensor_copy(dst[mt][:, qb * QB : (qb + 1) * QB], ps[:])

                  def qk_proj_block(x_r, w_t, dst, qb):
                      xt = qk_proj_dma(x_r, qb)
                      for mt in range(2):
                          qk_proj_mt(w_t, dst, qb, xt, mt)

                  def v_proj_block(qb):
                      xt = xpool.tile([P, NKC, QB], f32r, tag="xs", name="xt")
                      nc.sync.dma_start(xt[:], xv_r[:, :, qb * QB : (qb + 1) * QB])
                      for i in range(QB // P):
                          st = qb * (QB // P) + i
                          ps = ps_a.tile([P, GP], f32, tag="acc", name="ps")
                          for c in range(NKC):
                              nc.tensor.matmul(
                                  ps[:],
                                  lhsT=xt[:, c, i * P : (i + 1) * P],
                                  rhs=wv_s[:, c, :],
                                  start=(c == 0),
                                  stop=(c == NKC - 1),
                              )
                          nc.vector.tensor_copy(
                              V_sb[:, st, :, 0:DH],
                              ps[:].rearrange("p (h d) -> p h d", h=HG),
                          )
                          for h in range(HG):
                              nc.gpsimd.tensor_copy(
                                  V_sb[:, st, h, DH : DH + 1],
                                  mask_s[:, st : st + 1],
                              )

                  # ---- attention for one (q-block, head pair) ----
                  def attn_pair(qb, pr, chunks=None, chunk_cb=None):
                      """chunks/chunk_cb: optional callback after each 4-kt chunk
                      (used to weave the startup K/V projection blocks)."""
                      qsl = slice(qb * QB, (qb + 1) * QB)
                      ot_ps = [
                          ps_a.tile([DH + 1, QB], f32, tag="acc", name=f"otps{i}")
                          for i in range(2)
                      ]
                      prev = None
                      for kt in range(NKT):
                          s_ps = ps_s.tile([P, 2, QB], f32, tag="s", name="sps")
                          for hp in range(2):
                              h = 2 * pr + hp
                              hsl = slice(hp * DH, (hp + 1) * DH)
                              nc.tensor.matmul(
                                  s_ps[:, hp, :],
                                  lhsT=KT[pr][hsl, kt * P : (kt + 1) * P],
                                  rhs=QT[pr][hsl, qsl],
                                  start=True,
                                  stop=True,
                              )
                          pt = ppool.tile([P, 2, QB], f32r, tag="p", name="pt")
                          nc.scalar.activation(
                              pt[:], s_ps[:], bass.mybir.ActivationFunctionType.Exp
                          )
                          if prev is not None:
                              pprev, ktp = prev
                              for hp in range(2):
                                  nc.tensor.matmul(
                                      ot_ps[hp][:],
                                      lhsT=V_sb[:, ktp, 2 * pr + hp, :],
                                      rhs=pprev[:, hp, :],
                                      start=(ktp == 0),
                                      stop=False,
                                  )
                          prev = (pt, kt)
                          if chunk_cb is not None and kt % 4 == 3:
                              chunk_cb(kt // 4)
                          elif fillers and kt % 3 == 2:
                              fillers.pop(0)()
                      pprev, ktp = prev
                      for hp in range(2):
                          nc.tensor.matmul(
                              ot_ps[hp][:],
                              lhsT=V_sb[:, ktp, 2 * pr + hp, :],
                              rhs=pprev[:, hp, :],
                              start=(ktp == 0),
                              stop=True,
                          )
                      # normalize: reciprocal of denominators (psum row 64),
                      # move to partition 0 by DMA, broadcast on gpsimd, scale.
                      for hp in range(2):
                          rc = rcpool.tile([DH + 1, QB], f32r, tag="rc", name="rc")
                          with nc.allow_low_precision(reason="fp32r normalization"):
                              nc.vector.reciprocal(
                                  rc[DH : DH + 1, :], ot_ps[hp][DH : DH + 1, :]
                              )
                          rc0 = rcpool.tile([1, QB], f32r, tag="rc0", name="rc0")
                          nc.gpsimd.dma_start(rc0[0:1, :], rc[DH : DH + 1, :])
                          bc = bcpool.tile([P, QB], f32r, tag="bc", name="bc")
                          nc.gpsimd.partition_broadcast(bc[:], rc0[0:1, :])
                          if hp == 0:
                              nc.vector.tensor_copy(OT[pr][0:DH, qsl], ot_ps[hp][0:DH, :])
                              nc.vector.tensor_mul(
                                  OT[pr][0:DH, qsl], OT[pr][0:DH, qsl], bc[0:DH, :]
                              )
                          else:
                              oev = opool.tile([DH, QB], f32r, tag="oev", name="oev")
                              nc.vector.tensor_copy(oev[:], ot_ps[hp][0:DH, :])
                              nc.gpsimd.dma_start(OT[pr][DH:P, qsl], oev[:])
                              nc.vector.tensor_mul(
                                  OT[pr][DH:P, qsl], OT[pr][DH:P, qsl], bc[DH:P, :]
                              )

                  def wo_st(st):
                      ssl = slice(st * P, (st + 1) * P)
                      for nh in range(2):
                          ps = ps_a.tile([P, QB], f32, tag="acc", name="wops")
                          for t in range(2):
                              nc.tensor.matmul(
                                  ps[:],
                                  lhsT=OT[t][:, ssl],
                                  rhs=wo_s[:, t, nh * QB : (nh + 1) * QB],
                                  start=(t == 0),
                                  stop=(t == 1),
                              )
                          osb = opool.tile([P, QB], f32, tag="osb", name="osb")
                          nc.vector.tensor_copy(osb[:], ps[:])
                          nc.gpsimd.dma_start(outp[ssl, nh * QB : (nh + 1) * QB], osb[:])

                  # ---- emission ----
                  # startup: block 0 of K/V/Q, then weave remaining K/V blocks
                  # into q-block 0 / pair 0's kt loop.
                  qk_proj_block(xk_r, wk_s, KT, 0)
                  nc.sync.dma_start(wq_s[:], wq.rearrange("(c p) m -> p c m", p=P))
                  qk_proj_block(xq_r, wq_s, QT, 0)
                  nc.sync.dma_start(wv_s[:], wv.rearrange("(c p) m -> p c m", p=P))
                  nc.sync.dma_start(mask_s[:], maskd[:])
                  v_proj_block(0)
                  nc.sync.dma_start(wo_s[:], wo.rearrange("(c p) m -> p c m", p=P))

                  def startup_cb(b):
                      if b < NQB - 1:
                          qk_proj_block(xk_r, wk_s, KT, b + 1)
                          v_proj_block(b + 1)

                  def push_qproj(qb):
                      xt_n = qk_proj_dma(xq_r, qb)
                      for mt in range(2):
                          fillers.append(
                              lambda m=mt, xt=xt_n, q=qb: qk_proj_mt(wq_s, QT, q, xt, m)
                          )

                  attn_pair(0, 0, chunk_cb=startup_cb)
                  push_qproj(1)
                  attn_pair(0, 1)
                  for qb in range(1, NQB):
                      # fillers consumed during THIS q-block's loops must have
                      # been producible already: Wo of qb-1, Q proj of qb+1.
                      for st in range((qb - 1) * (QB // P), qb * (QB // P)):
                          fillers.append(lambda s=st: wo_st(s))
                      if qb + 1 < NQB:
                          push_qproj(qb + 1)
                      attn_pair(qb, 0)
                      attn_pair(qb, 1)
                      while fillers:
                          fillers.pop(0)()
                  for st in range((NQB - 1) * (QB // P), NQB * (QB // P)):
                      wo_st(st)

    nc.compile()
    _BUILD_CACHE[key] = nc
    return nc


def _prep_inputs(queries, keys, values, Wq, Wk, Wv, Wo, valid_seq_lens):
    qn = np.asarray(queries, dtype=np.float32)
    kn = np.asarray(keys, dtype=np.float32)
    vn = np.asarray(values, dtype=np.float32)
    wqn = np.asarray(Wq, dtype=np.float32) * np.float32(1.0 / np.sqrt(DH))
    wkn = np.asarray(Wk, dtype=np.float32)
    wvn = np.asarray(Wv, dtype=np.float32)
    won = np.asarray(Wo, dtype=np.float32)
    vl = np.asarray(valid_seq_lens).astype(np.int64)

    in_maps = []
    for core in range(8):
        b, g = core // 4, core % 4
        gsl = slice(g * GP, (g + 1) * GP)
        vmask = (np.arange(S) < vl[b]).astype(np.float32)
        vb = vn[b] * vmask[:, None]  # zero masked value rows
        in_maps.append(
            {
                "xq_t": np.ascontiguousarray(qn[b].T),
                "xk_t": np.ascontiguousarray(kn[b].T),
                "xv_t": np.ascontiguousarray(vb.T),
                "wq": np.ascontiguousarray(wqn[:, gsl]),
                "wk": np.ascontiguousarray(wkn[:, gsl]),
                "wv": np.ascontiguousarray(wvn[:, gsl]),
                "wo": np.ascontiguousarray(won[gsl, :]),
                "mask_t": np.ascontiguousarray(vmask.reshape(NKT, P).T),
            }
        )
    return in_maps


def kernel(queries, keys, values, Wq, Wk, Wv, Wo, valid_seq_lens):
    from concourse.bass_utils import run_bass_kernel_spmd

    nc = _build()
    in_maps = _prep_inputs(
        queries, keys, values, Wq, Wk, Wv, Wo, valid_seq_lens
    )
    res = run_bass_kernel_spmd(nc, in_maps, list(range(8)))
    out = np.zeros((B, S, D), dtype=np.float32)
    for core in range(8):
        out[core // 4] += res.results[core]["out_p"]
    return out



# revision 2
# speedup vs baseline: 1.1397x; 1.1397x over previous
"""Multi-head attention (B=2, S=2048, D=1024, H=16) on 8 Trainium2 cores.

Sharding: core c handles batch b=c//4 and head group g=c%4 (4 heads, 256 dims).
All activations/weights in bf16 (fp32 PSUM accumulation); rel-err gate 2e-2.
Only nkt = ceil(max(valid_seq_lens)/128) key tiles are computed — key tiles
beyond valid_len contribute exactly 0 (V rows zeroed + mask column) so they
are skipped at compile time (program is specialized on nkt, derived from the
actual inputs at call time).

Per-core math (all matmuls bf16 on the PE):
  Q^T = (Wq/8).T @ xq^T          [256, 2048]   (scale folded into Wq)
  K^T = Wk.T @ xk^T              [256, nkt*128]
  V   = xv @ Wv                  [nkt*128, 256] (xv rows >= valid_len zeroed on host)
  per head h, transposed scores S^T = K_h @ Q_h^T  [128k, 512q] per key tile;
  the head-pair's two score matmuls use PE row groups 0-1 / 2-3 (contraction
  rows 0:64 / 64:128) so they can execute concurrently on the array.
  p^T = exp(S^T)                 (no max subtraction; scores are O(10))
  [O_h^T; denom] = [V_h | mask].T @ p^T   (mask column gives the masked softmax
                                           denominator for free)
  O_h^T normalized by 1/denom (gpsimd partition_broadcast of reciprocal)
  out_partial = O @ Wo_g         [2048, 1024]
Host sums the 4 head-group partials per batch.
"""

import numpy as np
from contextlib import ExitStack

B, S, D, H = 2, 2048, 1024, 16
HG = 4  # heads per core
DH = 64
GP = HG * DH  # 256: per-core projection width
P = 128
QB = 512  # q block (matmul free dim)
NQB = S // QB
NKC = D // P  # 8 contraction chunks over D
NKT_MAX = S // P  # 16

_BUILD_CACHE = {}


def nkt_for(valid_seq_lens):
    vl = int(np.max(np.asarray(valid_seq_lens)))
    return max(1, min(NKT_MAX, (vl + P - 1) // P))


def _build(nkt=NKT_MAX, reps=1, loop_n=1):
    key = (nkt, reps, loop_n)
    if key in _BUILD_CACHE:
        return _BUILD_CACHE[key]
    import concourse.bass as bass
    import concourse.tile as tile
    from concourse import bacc, mybir

    f32 = mybir.dt.float32
    bf16 = mybir.dt.bfloat16

    nkb = (nkt + 3) // 4  # number of 512-wide key/value projection blocks
    kv_cols = nkt * P  # total key columns actually used

    nc = bacc.Bacc("TRN2", target_bir_lowering=False, debug=False, num_devices=8)

    xq = nc.dram_tensor("xq_t", [D, S], bf16, kind="ExternalInput").ap()
    xk = nc.dram_tensor("xk_t", [D, S], bf16, kind="ExternalInput").ap()
    xv = nc.dram_tensor("xv_t", [D, S], bf16, kind="ExternalInput").ap()
    wq = nc.dram_tensor("wq", [D, GP], bf16, kind="ExternalInput").ap()
    wk = nc.dram_tensor("wk", [D, GP], bf16, kind="ExternalInput").ap()
    wv = nc.dram_tensor("wv", [D, GP], bf16, kind="ExternalInput").ap()
    wo = nc.dram_tensor("wo", [GP, D], bf16, kind="ExternalInput").ap()
    maskd = nc.dram_tensor("mask_t", [P, NKT_MAX], bf16, kind="ExternalInput").ap()
    outp = nc.dram_tensor("out_p", [S, D], f32, kind="ExternalOutput").ap()

    with tile.TileContext(nc) as tc:
        with ExitStack() as ctx:
            wpool = ctx.enter_context(tc.tile_pool(name="weights", bufs=1))
            xpool = ctx.enter_context(tc.tile_pool(name="xstream", bufs=3))
            qkpool = ctx.enter_context(tc.tile_pool(name="qk", bufs=1))
            vpool = ctx.enter_context(tc.tile_pool(name="v", bufs=1))
            otpool = ctx.enter_context(tc.tile_pool(name="ot", bufs=1))
            ppool = ctx.enter_context(tc.tile_pool(name="p", bufs=4))
            rcpool = ctx.enter_context(tc.tile_pool(name="rc", bufs=4))
            bcpool = ctx.enter_context(tc.tile_pool(name="bc", bufs=4))
            opool = ctx.enter_context(tc.tile_pool(name="oev", bufs=4))
            # PSUM: 4 one-bank slots (2 held by the live attention pair,
            # 2 rotating for projections / output) + 2 two-bank score slots.
            ps_a = ctx.enter_context(tc.tile_pool(name="ps_a", bufs=4, space="PSUM"))
            ps_s = ctx.enter_context(tc.tile_pool(name="ps_s", bufs=2, space="PSUM"))

            # ---- constants / weights ----
            wq_s = wpool.tile([P, NKC, GP], bf16, tag="wq")
            wk_s = wpool.tile([P, NKC, GP], bf16, tag="wk")
            wv_s = wpool.tile([P, NKC, GP], bf16, tag="wv")
            wo_s = wpool.tile([P, GP // P, D], bf16, tag="wo")
            mask_s = wpool.tile([P, NKT_MAX], bf16, tag="mask")
            nc.sync.dma_start(wk_s[:], wk.rearrange("(c p) m -> p c m", p=P))
            nc.sync.dma_start(wq_s[:], wq.rearrange("(c p) m -> p c m", p=P))
            nc.sync.dma_start(wv_s[:], wv.rearrange("(c p) m -> p c m", p=P))
            nc.sync.dma_start(wo_s[:], wo.rearrange("(c p) m -> p c m", p=P))
            nc.sync.dma_start(mask_s[:], maskd[:])

            xq_r = xq.rearrange("(c p) s -> p c s", p=P)
            xk_r = xk.rearrange("(c p) s -> p c s", p=P)
            xv_r = xv.rearrange("(c p) s -> p c s", p=P)

            from contextlib import nullcontext
            for _ in range(reps):
              with (tc.For_i(0, loop_n, 1) if loop_n > 1 else nullcontext()):
                  # ---- residents ----
                  QT = [qkpool.tile([P, S], bf16, tag=f"qt{t}", name=f"qt{t}") for t in range(2)]
                  KT = [qkpool.tile([P, S], bf16, tag=f"kt{t}", name=f"kt{t}") for t in range(2)]
                  V_sb = vpool.tile([P, NKT_MAX, HG, DH + 1], bf16, tag="vsb")
                  OT = [otpool.tile([P, S], bf16, tag=f"ot{t}", name=f"ot{t}") for t in range(2)]

                  # mask columns of V (denominator trick), all key tiles at once
                  for h in range(HG):
                      nc.gpsimd.tensor_copy(
                          V_sb[:, 0:nkt, h, DH : DH + 1], mask_s[:, 0:nkt]
                      )

                  fillers = []  # small PE work units woven into the kt loops

                  def qk_proj_dma(x_r, qb, cols=QB):
                      xt = xpool.tile([P, NKC, QB], bf16, tag="xs", name="xt")
                      nc.sync.dma_start(
                          xt[:, :, 0:cols], x_r[:, :, qb * QB : qb * QB + cols]
                      )
                      return xt

                  def qk_proj_mt(w_t, dst, qb, xt, mt, cols=QB):
                      ps = ps_a.tile([P, QB], f32, tag="acc", name="ps")
                      for c in range(NKC):
                          nc.tensor.matmul(
                              ps[:, 0:cols],
                              lhsT=w_t[:, c, mt * P : (mt + 1) * P],
                              rhs=xt[:, c, 0:cols],
                              start=(c == 0),
                              stop=(c == NKC - 1),
                          )
                      nc.vector.tensor_copy(
                          dst[mt][:, qb * QB : qb * QB + cols], ps[:, 0:cols]
                      )

                  def qk_proj_block(x_r, w_t, dst, qb, cols=QB):
                      xt = qk_proj_dma(x_r, qb, cols)
                      for mt in range(2):
                          qk_proj_mt(w_t, dst, qb, xt, mt, cols)

                  def v_proj_block(qb, nst=QB // P):
                      xt = xpool.tile([P, NKC, QB], bf16, tag="xs", name="xt")
                      nc.sync.dma_start(
                          xt[:, :, 0 : nst * P],
                          xv_r[:, :, qb * QB : qb * QB + nst * P],
                      )
                      for i in range(nst):
                          st = qb * (QB // P) + i
                          ps = ps_a.tile([P, GP], f32, tag="acc", name="ps")
                          for c in range(NKC):
                              nc.tensor.matmul(
                                  ps[:],
                                  lhsT=xt[:, c, i * P : (i + 1) * P],
                                  rhs=wv_s[:, c, :],
                                  start=(c == 0),
                                  stop=(c == NKC - 1),
                              )
                          nc.vector.tensor_copy(
                              V_sb[:, st, :, 0:DH],
                              ps[:].rearrange("p (h d) -> p h d", h=HG),
                          )

                  # ---- attention for one (q-block, head pair) ----
                  def attn_pair(qb, pr, chunk_cb=None):
                      """chunk_cb: optional callback after each 4-kt chunk
                      (used to weave the startup K/V projection blocks)."""
                      qsl = slice(qb * QB, (qb + 1) * QB)
                      ot_ps = [
                          ps_a.tile([DH + 1, QB], f32, tag="acc", name=f"otps{i}")
                          for i in range(2)
                      ]
                      prev = None
                      for kt in range(nkt):
                          s_ps = ps_s.tile([P, 2, QB], f32, tag="s", name="sps")
                          for hp in range(2):
                              hsl = slice(hp * DH, (hp + 1) * DH)
                              nc.tensor.matmul(
                                  s_ps[:, hp, :],
                                  lhsT=KT[pr][hsl, kt * P : (kt + 1) * P],
                                  rhs=QT[pr][hsl, qsl],
                                  start=True,
                                  stop=True,
                              )
                          pt = ppool.tile([P, 2, QB], bf16, tag="p", name="pt")
                          nc.scalar.activation(
                              pt[:], s_ps[:], bass.mybir.ActivationFunctionType.Exp
                          )
                          if prev is not None:
                              pprev, ktp = prev
                              for hp in range(2):
                                  nc.tensor.matmul(
                                      ot_ps[hp][:],
                                      lhsT=V_sb[:, ktp, 2 * pr + hp, :],
                                      rhs=pprev[:, hp, :],
                                      start=(ktp == 0),
                                      stop=False,
                                  )
                          prev = (pt, kt)
                          if chunk_cb is not None and kt % 4 == 3:
                              chunk_cb(kt // 4)
                          elif fillers and kt % 3 == 2:
                              fillers.pop(0)()
                      pprev, ktp = prev
                      for hp in range(2):
                          nc.tensor.matmul(
                              ot_ps[hp][:],
                              lhsT=V_sb[:, ktp, 2 * pr + hp, :],
                              rhs=pprev[:, hp, :],
                              start=(ktp == 0),
                              stop=True,
                          )
                      # normalize: reciprocal of denominators (psum row 64),
                      # move to partition 0 by DMA, broadcast on gpsimd, scale.
                      for hp in range(2):
                          rc = rcpool.tile([DH + 1, QB], bf16, tag="rc", name="rc")
                          with nc.allow_low_precision(reason="bf16 normalization"):
                              nc.vector.reciprocal(
                                  rc[DH : DH + 1, :], ot_ps[hp][DH : DH + 1, :]
                              )
                          rc0 = rcpool.tile([1, QB], bf16, tag="rc0", name="rc0")
                          nc.sync.dma_start(rc0[0:1, :], rc[DH : DH + 1, :])
                          bc = bcpool.tile([P, QB], bf16, tag="bc", name="bc")
                          nc.gpsimd.partition_broadcast(bc[:], rc0[0:1, :])
                          with nc.allow_low_precision(reason="bf16 normalization"):
                              if hp == 0:
                                  nc.vector.tensor_copy(
                                      OT[pr][0:DH, qsl], ot_ps[hp][0:DH, :]
                                  )
                                  nc.vector.tensor_mul(
                                      OT[pr][0:DH, qsl], OT[pr][0:DH, qsl], bc[0:DH, :]
                                  )
                              else:
                                  oev = opool.tile([DH, QB], bf16, tag="oev", name="oev")
                                  nc.vector.tensor_copy(oev[:], ot_ps[hp][0:DH, :])
                                  nc.sync.dma_start(OT[pr][DH:P, qsl], oev[:])
                                  nc.vector.tensor_mul(
                                      OT[pr][DH:P, qsl], OT[pr][DH:P, qsl], bc[DH:P, :]
                                  )

                  def wo_st(st):
                      ssl = slice(st * P, (st + 1) * P)
                      for nh in range(2):
                          ps = ps_a.tile([P, QB], f32, tag="acc", name="wops")
                          for t in range(2):
                              nc.tensor.matmul(
                                  ps[:],
                                  lhsT=OT[t][:, ssl],
                                  rhs=wo_s[:, t, nh * QB : (nh + 1) * QB],
                                  start=(t == 0),
                                  stop=(t == 1),
                              )
                          osb = opool.tile([P, QB], f32, tag="osb", name="osb")
                          nc.vector.tensor_copy(osb[:], ps[:])
                          nc.sync.dma_start(outp[ssl, nh * QB : (nh + 1) * QB], osb[:])

                  def kv_block_cols(b):
                      # columns of key/value projection block b actually needed
                      return min(QB, kv_cols - b * QB)

                  # ---- emission ----
                  # startup: block 0 of K/V/Q, then weave remaining K/V blocks
                  # into q-block 0 / pair 0's kt loop.
                  qk_proj_block(xk_r, wk_s, KT, 0, kv_block_cols(0))
                  qk_proj_block(xq_r, wq_s, QT, 0)
                  v_proj_block(0, min(QB // P, nkt))

                  def startup_cb(b):
                      if b + 1 < nkb:
                          cols = kv_block_cols(b + 1)
                          qk_proj_block(xk_r, wk_s, KT, b + 1, cols)
                          v_proj_block(b + 1, (cols + P - 1) // P)
                      elif fillers:
                          fillers.pop(0)()

                  def push_qproj(qb):
                      xt_n = qk_proj_dma(xq_r, qb)
                      for mt in range(2):
                          fillers.append(
                              lambda m=mt, xt=xt_n, q=qb: qk_proj_mt(wq_s, QT, q, xt, m)
                          )

                  attn_pair(0, 0, chunk_cb=startup_cb)
                  push_qproj(1)
                  attn_pair(0, 1)
                  for qb in range(1, NQB):
                      # fillers consumed during THIS q-block's loops must have
                      # been producible already: Wo of qb-1, Q proj of qb+1.
                      for st in range((qb - 1) * (QB // P), qb * (QB // P)):
                          fillers.append(lambda s=st: wo_st(s))
                      if qb + 1 < NQB:
                          push_qproj(qb + 1)
                      attn_pair(qb, 0)
                      attn_pair(qb, 1)
                      while fillers:
                          fillers.pop(0)()
                  for st in range((NQB - 1) * (QB // P), NQB * (QB // P)):
                      wo_st(st)

    nc.compile()
    _BUILD_CACHE[key] = nc
    return nc


def _prep_inputs(queries, keys, values, Wq, Wk, Wv, Wo, valid_seq_lens):
    import ml_dtypes

    bf = ml_dtypes.bfloat16
    qn = np.asarray(queries, dtype=np.float32)
    kn = np.asarray(keys, dtype=np.float32)
    vn = np.asarray(values, dtype=np.float32)
    wqn = (np.asarray(Wq, dtype=np.float32) * np.float32(1.0 / np.sqrt(DH))).astype(bf)
    wkn = np.asarray(Wk, dtype=np.float32).astype(bf)
    wvn = np.asarray(Wv, dtype=np.float32).astype(bf)
    won = np.asarray(Wo, dtype=np.float32).astype(bf)
    vl = np.asarray(valid_seq_lens).astype(np.int64)

    in_maps = []
    for core in range(8):
        b, g = core // 4, core % 4
        gsl = slice(g * GP, (g + 1) * GP)
        vmask = (np.arange(S) < vl[b]).astype(np.float32)
        vb = vn[b] * vmask[:, None]  # zero masked value rows
        in_maps.append(
            {
                "xq_t": np.ascontiguousarray(qn[b].T.astype(bf)),
                "xk_t": np.ascontiguousarray(kn[b].T.astype(bf)),
                "xv_t": np.ascontiguousarray(vb.T.astype(bf)),
                "wq": np.ascontiguousarray(wqn[:, gsl]),
                "wk": np.ascontiguousarray(wkn[:, gsl]),
                "wv": np.ascontiguousarray(wvn[:, gsl]),
                "wo": np.ascontiguousarray(won[gsl, :]),
                "mask_t": np.ascontiguousarray(
                    vmask.reshape(NKT_MAX, P).T.astype(bf)
                ),
            }
        )
    return in_maps


def kernel(queries, keys, values, Wq, Wk, Wv, Wo, valid_seq_lens):
    from concourse.bass_utils import run_bass_kernel_spmd

    nc = _build(nkt=nkt_for(valid_seq_lens))
    in_maps = _prep_inputs(
        queries, keys, values, Wq, Wk, Wv, Wo, valid_seq_lens
    )
    res = run_bass_kernel_spmd(nc, in_maps, list(range(8)))
    out = np.zeros((B, S, D), dtype=np.float32)
    for core in range(8):
        out[core // 4] += res.results[core]["out_p"]
    return out


# revision 18
# speedup vs baseline: 1.7414x; 1.5279x over previous
"""Multi-head attention (B=2, S=2048, D=1024, H=16) on 8 Trainium2 cores.

Sharding: core c handles batch b=c//4 and head group g=c%4 (4 heads, 256 dims).
All activations/weights in bf16 (fp32 PSUM accumulation); rel-err gate 2e-2.
Only nkt = ceil(max(valid_seq_lens)/128) key tiles are computed — key tiles
beyond valid_len contribute exactly 0 (V rows zeroed + mask column) so they
are skipped at compile time (program specialized on nkt, derived from the
actual inputs at call time).

PE strategy: the moving-operand SBUF path streams 128 bf16 elements/cycle,
so a K=128 matmul runs at 1 col/cycle while a K=64 matmul runs at 2 cols/
cycle — and two K=64 matmuls on disjoint row groups (partitions 0:64 vs
64:128) execute concurrently. Every contraction is therefore split into
half-K pairs on opposite row groups:
  scores: head-pair members live on partitions 0:64 / 64:128 (concurrent)
  AV:     key-halves of the 128-key tile, crossed over the two heads
  Q/K/V/Wo projections: contraction chunks split lo/hi, crossed over the
          two output tiles
Concurrent matmuls always write different PSUM banks; accumulation order
within a bank follows program order (PE completion is pc-monotone).

  p^T = exp(S^T) on the Activation engine (the throughput wall: ~1.25us per
  [128, 2x512] tile); the mask column appended to V yields the masked
  softmax denominator within the same AV accumulation.
Host sums the 4 head-group partials per batch.
"""

import numpy as np
from contextlib import ExitStack

B, S, D, H = 2, 2048, 1024, 16
HG = 4  # heads per core
DH = 64
GP = HG * DH  # 256: per-core projection width
P = 128
QB = 512  # q block (matmul free dim)
NQB = S // QB
NKC = D // P  # 8 contraction chunks over D
NKT_MAX = S // P  # 16

LO = slice(0, 64)
HI = slice(64, 128)

# Row-group-crossed K=64 split matmuls hang the device when accumulation
# groups alternate tile_position within a PSUM bank — keep serial (default).
import os
SPLIT_AV = os.environ.get("K_SPLIT_AV", "0") == "1"
SPLIT_PROJ = os.environ.get("K_SPLIT_PROJ", "0") == "1"

_BUILD_CACHE = {}


def nkt_for(valid_seq_lens):
    vl = int(np.max(np.asarray(valid_seq_lens)))
    return max(1, min(NKT_MAX, (vl + P - 1) // P))


def _build(nkt=NKT_MAX, reps=1, loop_n=1):
    key = (nkt, reps, loop_n)
    if key in _BUILD_CACHE:
        return _BUILD_CACHE[key]
    import concourse.bass as bass
    import concourse.tile as tile
    from concourse import bacc, mybir

    f32 = mybir.dt.float32
    bf16 = mybir.dt.bfloat16

    nkb = (nkt + 3) // 4  # number of 512-wide key/value projection blocks
    kv_cols = nkt * P  # total key columns actually used

    nc = bacc.Bacc("TRN2", target_bir_lowering=False, debug=False, num_devices=8)

    xq = nc.dram_tensor("xq_t", [D, S], bf16, kind="ExternalInput").ap()
    xk = nc.dram_tensor("xk_t", [D, S], bf16, kind="ExternalInput").ap()
    xv = nc.dram_tensor("xv_t", [D, S], bf16, kind="ExternalInput").ap()
    wq = nc.dram_tensor("wq", [D, GP], bf16, kind="ExternalInput").ap()
    wk = nc.dram_tensor("wk", [D, GP], bf16, kind="ExternalInput").ap()
    wv = nc.dram_tensor("wv", [D, GP], bf16, kind="ExternalInput").ap()
    wo = nc.dram_tensor("wo", [GP, D], bf16, kind="ExternalInput").ap()
    maskd = nc.dram_tensor("mask_t", [P, NKT_MAX], bf16, kind="ExternalInput").ap()
    outp = nc.dram_tensor("out_p", [S, D], f32, kind="ExternalOutput").ap()

    with tile.TileContext(nc) as tc:
        with ExitStack() as ctx:
            wpool = ctx.enter_context(tc.tile_pool(name="weights", bufs=1))
            xpool = ctx.enter_context(tc.tile_pool(name="xstream", bufs=8))
            qkpool = ctx.enter_context(tc.tile_pool(name="qk", bufs=1))
            vpool = ctx.enter_context(tc.tile_pool(name="v", bufs=1))
            otpool = ctx.enter_context(tc.tile_pool(name="ot", bufs=1))
            ppool = ctx.enter_context(tc.tile_pool(name="p", bufs=4))
            rcpool = ctx.enter_context(tc.tile_pool(name="rc", bufs=4))
            bcpool = ctx.enter_context(tc.tile_pool(name="bc", bufs=4))
            opool = ctx.enter_context(tc.tile_pool(name="oev", bufs=4))
            # PSUM (8 banks): "s" 2x2 banks (score tiles), "ot" 2x1 bank
            # (AV accumulators, live per attn_pair), "acc" 2x1 bank
            # (rotating projection / Wo accumulators).
            ps_p = ctx.enter_context(tc.tile_pool(name="ps_p", bufs=2, space="PSUM"))
            ps_s = ctx.enter_context(tc.tile_pool(name="ps_s", bufs=2, space="PSUM"))

            # ---- constants / weights ----
            wq_s = wpool.tile([P, NKC, GP], bf16, tag="wq")
            wk_s = wpool.tile([P, NKC, GP], bf16, tag="wk")
            wv_s = wpool.tile([P, NKC, GP], bf16, tag="wv")
            wo_s = wpool.tile([P, GP // P, D], bf16, tag="wo")
            mask_s = wpool.tile([P, NKT_MAX], bf16, tag="mask")
            nc.sync.dma_start(wk_s[:], wk.rearrange("(c p) m -> p c m", p=P))
            nc.sync.dma_start(wq_s[:], wq.rearrange("(c p) m -> p c m", p=P))
            nc.sync.dma_start(wv_s[:], wv.rearrange("(c p) m -> p c m", p=P))
            nc.sync.dma_start(mask_s[:], maskd[:])
            nc.sync.dma_start(wo_s[:], wo.rearrange("(c p) m -> p c m", p=P))

            xq_r = xq.rearrange("(c p) s -> p c s", p=P)
            xk_r = xk.rearrange("(c p) s -> p c s", p=P)
            xv_r = xv.rearrange("(c p) s -> p c s", p=P)

            from contextlib import nullcontext
            for _ in range(reps):
              with (tc.For_i(0, loop_n, 1) if loop_n > 1 else nullcontext()):
                  # ---- residents ----
                  QT = [qkpool.tile([P, S], bf16, tag=f"qt{t}", name=f"qt{t}") for t in range(2)]
                  KT = [qkpool.tile([P, S], bf16, tag=f"kt{t}", name=f"kt{t}") for t in range(2)]
                  V_sb = vpool.tile([P, NKT_MAX, HG, DH + 1], bf16, tag="vsb")
                  OT = [otpool.tile([P, S], bf16, tag=f"ot{t}", name=f"ot{t}") for t in range(2)]

                  # mask columns of V (denominator trick), all key tiles at once
                  for h in range(HG):
                      nc.gpsimd.tensor_copy(
                          V_sb[:, 0:nkt, h, DH : DH + 1], mask_s[:, 0:nkt]
                      )

                  # (deadline, closure) units woven into the kt loops.
                  # A unit with deadline D is guaranteed to be emitted by the
                  # top of attention iteration D-1 (before that iteration's
                  # lookahead scores), regardless of queue position.
                  fillers = []
                  FAR = 10**9

                  def x_dma(x_r, qb, cols=QB):
                      xt = xpool.tile([P, NKC, QB], bf16, tag="xs", name="xt")
                      nc.sync.dma_start(
                          xt[:, :, 0:cols], x_r[:, :, qb * QB : qb * QB + cols]
                      )
                      return xt

                  def qk_proj_half(w_t, dst, qb, xt, crange, cols=QB):
                      """Half of a projection block: contraction chunks crange,
                      both output tiles, as row-group-crossed K=64 pairs."""
                      if crange[0] == 0:
                          ps = [ps_p.tile([P, QB], f32, tag="acc", name=f"pj{m}",
                                          bufs=2) for m in range(2)]
                          qk_proj_half.ps = ps
                      else:
                          ps = qk_proj_half.ps
                      last = NKC - 1
                      for c in crange:
                          en = c == last
                          if SPLIT_PROJ:
                              nc.tensor.matmul(ps[0][:, 0:cols], lhsT=w_t[LO, c, 0:P],
                                               rhs=xt[LO, c, 0:cols],
                                               start=(c == 0), stop=False)
                              nc.tensor.matmul(ps[1][:, 0:cols], lhsT=w_t[HI, c, P:GP],
                                               rhs=xt[HI, c, 0:cols],
                                               start=(c == 0), stop=False)
                              nc.tensor.matmul(ps[1][:, 0:cols], lhsT=w_t[LO, c, P:GP],
                                               rhs=xt[LO, c, 0:cols],
                                               start=False, stop=en)
                              nc.tensor.matmul(ps[0][:, 0:cols], lhsT=w_t[HI, c, 0:P],
                                               rhs=xt[HI, c, 0:cols],
                                               start=False, stop=en)
                          else:
                              for mt in range(2):
                                  nc.tensor.matmul(
                                      ps[mt][:, 0:cols],
                                      lhsT=w_t[:, c, mt * P : (mt + 1) * P],
                                      rhs=xt[:, c, 0:cols],
                                      start=(c == 0), stop=en)
                      if crange[-1] == last:
                          for mt in range(2):
                              nc.vector.tensor_copy(
                                  dst[mt][:, qb * QB : qb * QB + cols],
                                  ps[mt][:, 0:cols],
                              )

                  def push_qk_proj(x_r, w_t, dst, qb, cols=QB, deadline=FAR):
                      xt = x_dma(x_r, qb, cols)
                      fillers.append((deadline,
                                      lambda: qk_proj_half(w_t, dst, qb, xt,
                                                           range(0, 4), cols)))
                      fillers.append((deadline,
                                      lambda: qk_proj_half(w_t, dst, qb, xt,
                                                           range(4, NKC), cols)))

                  def v_proj_pair(xt, qb, i0, two):
                      """V projection for st pair (i0, i0+1) as crossed K=64
                      pairs, or a single serial st when two=False."""
                      st0 = qb * (QB // P) + i0
                      if two:
                          ps = [ps_p.tile([P, GP], f32, tag="acc", name=f"vp{j}",
                                          bufs=2) for j in range(2)]
                          for c in range(NKC):
                              st_, en = (c == 0), (c == NKC - 1)
                              nc.tensor.matmul(ps[0][:], lhsT=xt[LO, c, i0 * P:(i0 + 1) * P],
                                               rhs=wv_s[LO, c, :], start=st_, stop=False)
                              nc.tensor.matmul(ps[1][:], lhsT=xt[HI, c, (i0 + 1) * P:(i0 + 2) * P],
                                               rhs=wv_s[HI, c, :], start=st_, stop=False)
                              nc.tensor.matmul(ps[1][:], lhsT=xt[LO, c, (i0 + 1) * P:(i0 + 2) * P],
                                               rhs=wv_s[LO, c, :], start=False, stop=en)
                              nc.tensor.matmul(ps[0][:], lhsT=xt[HI, c, i0 * P:(i0 + 1) * P],
                                               rhs=wv_s[HI, c, :], start=False, stop=en)
                          for j in range(2):
                              nc.vector.tensor_copy(
                                  V_sb[:, st0 + j, :, 0:DH],
                                  ps[j][:].rearrange("p (h d) -> p h d", h=HG),
                              )
                      else:
                          ps = ps_p.tile([P, GP], f32, tag="acc", name="vp0", bufs=2)
                          for c in range(NKC):
                              nc.tensor.matmul(ps[:], lhsT=xt[:, c, i0 * P:(i0 + 1) * P],
                                               rhs=wv_s[:, c, :],
                                               start=(c == 0), stop=(c == NKC - 1))
                          nc.vector.tensor_copy(
                              V_sb[:, st0, :, 0:DH],
                              ps[:].rearrange("p (h d) -> p h d", h=HG),
                          )

                  def push_v_proj(qb, nst):
                      xt = x_dma(xv_r, qb, nst * P)
                      i = 0
                      while i < nst:
                          two = SPLIT_PROJ and i + 1 < nst
                          st = qb * (QB // P) + i
                          fillers.append((min(st + 2, nkt),
                                          lambda x=xt, q=qb, i0=i, t=two:
                                          v_proj_pair(x, q, i0, t)))
                          i += 2 if two else 1

                  # ---- attention for one (q-block, head pair) ----
                  def attn_pair(qb, pr):
                      qsl = slice(qb * QB, (qb + 1) * QB)
                      ot_ps = [
                          ps_p.tile([DH + 1, QB], f32, tag="ot", name=f"otps{i}",
                                    bufs=2)
                          for i in range(2)
                      ]
                      h0, h1 = 2 * pr, 2 * pr + 1

                      def av(pt, kt, en):
                          st_ = kt == 0
                          if SPLIT_AV:
                              nc.tensor.matmul(ot_ps[0][:], lhsT=V_sb[LO, kt, h0, :],
                                               rhs=pt[LO, 0, :], start=st_, stop=False)
                              nc.tensor.matmul(ot_ps[1][:], lhsT=V_sb[HI, kt, h1, :],
                                               rhs=pt[HI, 1, :], start=st_, stop=False)
                              nc.tensor.matmul(ot_ps[1][:], lhsT=V_sb[LO, kt, h1, :],
                                               rhs=pt[LO, 1, :], start=False, stop=en)
                              nc.tensor.matmul(ot_ps[0][:], lhsT=V_sb[HI, kt, h0, :],
                                               rhs=pt[HI, 0, :], start=False, stop=en)
                          else:
                              for hp in range(2):
                                  nc.tensor.matmul(
                                      ot_ps[hp][:],
                                      lhsT=V_sb[:, kt, 2 * pr + hp, :],
                                      rhs=pt[:, hp, :], start=st_, stop=en)

                      def scores(kt):
                          s_ps = ps_s.tile([P, 2, QB], f32, tag="s", name="sps")
                          for hp in range(2):
                              hsl = slice(hp * DH, (hp + 1) * DH)
                              nc.tensor.matmul(
                                  s_ps[:, hp, :],
                                  lhsT=KT[pr][hsl, kt * P : (kt + 1) * P],
                                  rhs=QT[pr][hsl, qsl],
                                  start=True,
                                  stop=True,
                              )
                          return s_ps

                      # scores run one tile ahead of exp so the Act engine
                      # never queues behind the current iteration's PE work
                      prev = None
                      s_cur = scores(0)
                      for kt in range(nkt):
                          # fillers first: pop one unit per iteration, plus
                          # however many are needed to honor deadlines before
                          # this iteration's lookahead scores / next av
                          popped = False
                          while fillers and (
                              not popped
                              or min(dl for dl, _ in fillers) <= kt + 1
                          ):
                              fillers.pop(0)[1]()
                              popped = True
                          s_next = scores(kt + 1) if kt + 1 < nkt else None
                          pt = ppool.tile([P, 2, QB], bf16, tag="p", name="pt")
                          nc.scalar.activation(
                              pt[:], s_cur[:], bass.mybir.ActivationFunctionType.Exp
                          )
                          s_cur = s_next
                          if prev is not None:
                              av(prev, kt - 1, False)
                          prev = pt
                      av(prev, nkt - 1, True)
                      # normalize: reciprocal of denominators (psum row 64),
                      # move to partition 0 by DMA, broadcast on gpsimd, scale.
                      for hp in range(2):
                          rc = rcpool.tile([DH + 1, QB], bf16, tag="rc", name="rc")
                          with nc.allow_low_precision(reason="bf16 normalization"):
                              nc.vector.reciprocal(
                                  rc[DH : DH + 1, :], ot_ps[hp][DH : DH + 1, :]
                              )
                          rc0 = rcpool.tile([1, QB], bf16, tag="rc0", name="rc0")
                          nc.sync.dma_start(rc0[0:1, :], rc[DH : DH + 1, :])
                          bc = bcpool.tile([P, QB], bf16, tag="bc", name="bc")
                          nc.gpsimd.partition_broadcast(bc[:], rc0[0:1, :])
                          with nc.allow_low_precision(reason="bf16 normalization"):
                              if hp == 0:
                                  nc.vector.tensor_copy(
                                      OT[pr][0:DH, qsl], ot_ps[hp][0:DH, :]
                                  )
                                  nc.vector.tensor_mul(
                                      OT[pr][0:DH, qsl], OT[pr][0:DH, qsl], bc[0:DH, :]
                                  )
                              else:
                                  oev = opool.tile([DH, QB], bf16, tag="oev", name="oev")
                                  nc.vector.tensor_copy(oev[:], ot_ps[hp][0:DH, :])
                                  nc.sync.dma_start(OT[pr][DH:P, qsl], oev[:])
                                  nc.vector.tensor_mul(
                                      OT[pr][DH:P, qsl], OT[pr][DH:P, qsl], bc[DH:P, :]
                                  )

                  def wo_st(st):
                      ssl = slice(st * P, (st + 1) * P)
                      ps = [ps_p.tile([P, QB], f32, tag="acc", name=f"wo{n}", bufs=2)
                            for n in range(2)]
                      for t in range(2):
                          st_, en = (t == 0), (t == 1)
                          if SPLIT_PROJ:
                              nc.tensor.matmul(ps[0][:], lhsT=OT[t][LO, ssl],
                                               rhs=wo_s[LO, t, 0:QB], start=st_, stop=False)
                              nc.tensor.matmul(ps[1][:], lhsT=OT[t][HI, ssl],
                                               rhs=wo_s[HI, t, QB:D], start=st_, stop=False)
                              nc.tensor.matmul(ps[1][:], lhsT=OT[t][LO, ssl],
                                               rhs=wo_s[LO, t, QB:D], start=False, stop=en)
                              nc.tensor.matmul(ps[0][:], lhsT=OT[t][HI, ssl],
                                               rhs=wo_s[HI, t, 0:QB], start=False, stop=en)
                          else:
                              for nh in range(2):
                                  nc.tensor.matmul(
                                      ps[nh][:], lhsT=OT[t][:, ssl],
                                      rhs=wo_s[:, t, nh * QB : (nh + 1) * QB],
                                      start=st_, stop=en)
                      for nh in range(2):
                          osb = opool.tile([P, QB], f32, tag="osb", name="osb")
                          nc.vector.tensor_copy(osb[:], ps[nh][:])
                          nc.sync.dma_start(outp[ssl, nh * QB : (nh + 1) * QB], osb[:])

                  def kv_block_cols(b):
                      return min(QB, kv_cols - b * QB)

                  # ---- emission ----
                  # startup: block 0 of K/Q/V computed inline, then the
                  # remaining K/V blocks woven one filler unit per kt of the
                  # first attention pass.
                  xt_k0 = x_dma(xk_r, 0, kv_block_cols(0))
                  xt_q0 = x_dma(xq_r, 0)
                  qk_proj_half(wk_s, KT, 0, xt_k0, range(0, 4), kv_block_cols(0))
                  qk_proj_half(wk_s, KT, 0, xt_k0, range(4, NKC), kv_block_cols(0))
                  qk_proj_half(wq_s, QT, 0, xt_q0, range(0, 4))
                  qk_proj_half(wq_s, QT, 0, xt_q0, range(4, NKC))
                  push_v_proj(0, min(QB // P, nkt))
                  for b in range(1, nkb):
                      cols = kv_block_cols(b)
                      push_qk_proj(xk_r, wk_s, KT, b, cols, deadline=4 * b)
                      push_v_proj(b, (cols + P - 1) // P)

                  attn_pair(0, 0)
                  push_qk_proj(xq_r, wq_s, QT, 1)
                  attn_pair(0, 1)
                  for qb in range(1, NQB):
                      # fillers consumed during THIS q-block's loops must have
                      # been producible already: Wo of qb-1, Q proj of qb+1.
                      for st in range((qb - 1) * (QB // P), qb * (QB // P)):
                          fillers.append((FAR, lambda s=st: wo_st(s)))
                      if qb + 1 < NQB:
                          push_qk_proj(xq_r, wq_s, QT, qb + 1)
                      attn_pair(qb, 0)
                      attn_pair(qb, 1)
                      while fillers:
                          fillers.pop(0)[1]()
                  for st in range((NQB - 1) * (QB // P), NQB * (QB // P)):
                      wo_st(st)

    nc.compile()
    _BUILD_CACHE[key] = nc
    return nc


def _prep_inputs(queries, keys, values, Wq, Wk, Wv, Wo, valid_seq_lens):
    import ml_dtypes

    bf = ml_dtypes.bfloat16
    qn = np.asarray(queries, dtype=np.float32)
    kn = np.asarray(keys, dtype=np.float32)
    vn = np.asarray(values, dtype=np.float32)
    wqn = (np.asarray(Wq, dtype=np.float32) * np.float32(1.0 / np.sqrt(DH))).astype(bf)
    wkn = np.asarray(Wk, dtype=np.float32).astype(bf)
    wvn = np.asarray(Wv, dtype=np.float32).astype(bf)
    won = np.asarray(Wo, dtype=np.float32).astype(bf)
    vl = np.asarray(valid_seq_lens).astype(np.int64)

    in_maps = []
    for core in range(8):
        b, g = core // 4, core % 4
        gsl = slice(g * GP, (g + 1) * GP)
        vmask = (np.arange(S) < vl[b]).astype(np.float32)
        vb = vn[b] * vmask[:, None]  # zero masked value rows
        in_maps.append(
            {
                "xq_t": np.ascontiguousarray(qn[b].T.astype(bf)),
                "xk_t": np.ascontiguousarray(kn[b].T.astype(bf)),
                "xv_t": np.ascontiguousarray(vb.T.astype(bf)),
                "wq": np.ascontiguousarray(wqn[:, gsl]),
                "wk": np.ascontiguousarray(wkn[:, gsl]),
                "wv": np.ascontiguousarray(wvn[:, gsl]),
                "wo": np.ascontiguousarray(won[gsl, :]),
                "mask_t": np.ascontiguousarray(
                    vmask.reshape(NKT_MAX, P).T.astype(bf)
                ),
            }
        )
    return in_maps


def kernel(queries, keys, values, Wq, Wk, Wv, Wo, valid_seq_lens):
    from concourse.bass_utils import run_bass_kernel_spmd

    nc = _build(nkt=nkt_for(valid_seq_lens))
    in_maps = _prep_inputs(
        queries, keys, values, Wq, Wk, Wv, Wo, valid_seq_lens
    )
    res = run_bass_kernel_spmd(nc, in_maps, list(range(8)))
    out = np.zeros((B, S, D), dtype=np.float32)
    for core in range(8):
        out[core // 4] += res.results[core]["out_p"]
    return out


# revision 21
# speedup vs baseline: 1.8372x; 1.0551x over previous
"""Multi-head attention (B=2, S=2048, D=1024, H=16) on 8 Trainium2 cores.

Sharding: core c handles batch b=c//4 and head group g=c%4 (4 heads, 256 dims).
All activations/weights in bf16 (fp32 PSUM accumulation); rel-err gate 2e-2.
Only nkt = ceil(max(valid_seq_lens)/128) key tiles are computed — key tiles
beyond valid_len contribute exactly 0 (V rows zeroed + mask column) so they
are skipped at compile time (program specialized on nkt, derived from the
actual inputs at call time).

PE strategy: the moving-operand SBUF path streams 128 bf16 elements/cycle,
so a K=128 matmul runs at 1 col/cycle while a K=64 matmul runs at 2 cols/
cycle — and two K=64 matmuls on disjoint row groups (partitions 0:64 vs
64:128) execute concurrently. Every contraction is therefore split into
half-K pairs on opposite row groups:
  scores: head-pair members live on partitions 0:64 / 64:128 (concurrent)
  AV:     key-halves of the 128-key tile, crossed over the two heads
  Q/K/V/Wo projections: contraction chunks split lo/hi, crossed over the
          two output tiles
Concurrent matmuls always write different PSUM banks; accumulation order
within a bank follows program order (PE completion is pc-monotone).

  p^T = exp(S^T) on the Activation engine (the throughput wall: ~1.25us per
  [128, 2x512] tile); the mask column appended to V yields the masked
  softmax denominator within the same AV accumulation.
Host sums the 4 head-group partials per batch.
"""

import numpy as np
from contextlib import ExitStack

B, S, D, H = 2, 2048, 1024, 16
HG = 4  # heads per core
DH = 64
GP = HG * DH  # 256: per-core projection width
P = 128
QB = 512  # q block (matmul free dim)
NQB = S // QB
NKC = D // P  # 8 contraction chunks over D
NKT_MAX = S // P  # 16

LO = slice(0, 64)
HI = slice(64, 128)

# Row-group-crossed K=64 split matmuls hang the device when accumulation
# groups alternate tile_position within a PSUM bank — keep serial (default).
import os
SPLIT_AV = os.environ.get("K_SPLIT_AV", "0") == "1"
SPLIT_PROJ = os.environ.get("K_SPLIT_PROJ", "0") == "1"

_BUILD_CACHE = {}


def nkt_for(valid_seq_lens):
    vl = int(np.max(np.asarray(valid_seq_lens)))
    return max(1, min(NKT_MAX, (vl + P - 1) // P))


def _build(nkt=NKT_MAX, reps=1, loop_n=1):
    key = (nkt, reps, loop_n)
    if key in _BUILD_CACHE:
        return _BUILD_CACHE[key]
    import concourse.bass as bass
    import concourse.tile as tile
    from concourse import bacc, mybir

    f32 = mybir.dt.float32
    bf16 = mybir.dt.bfloat16

    nkb = (nkt + 3) // 4  # number of 512-wide key/value projection blocks
    kv_cols = nkt * P  # total key columns actually used

    nc = bacc.Bacc("TRN2", target_bir_lowering=False, debug=False, num_devices=8)

    xq = nc.dram_tensor("xq_t", [D, S], bf16, kind="ExternalInput").ap()
    xk = nc.dram_tensor("xk_t", [D, S], bf16, kind="ExternalInput").ap()
    xv = nc.dram_tensor("xv_t", [D, S], bf16, kind="ExternalInput").ap()
    wq = nc.dram_tensor("wq", [D, GP], bf16, kind="ExternalInput").ap()
    wk = nc.dram_tensor("wk", [D, GP], bf16, kind="ExternalInput").ap()
    wv = nc.dram_tensor("wv", [D, GP], bf16, kind="ExternalInput").ap()
    wo = nc.dram_tensor("wo", [GP, D], bf16, kind="ExternalInput").ap()
    maskd = nc.dram_tensor("mask_t", [P, NKT_MAX], bf16, kind="ExternalInput").ap()
    outp = nc.dram_tensor("out_p", [S, D], f32, kind="ExternalOutput").ap()

    with tile.TileContext(nc) as tc:
        with ExitStack() as ctx:
            wpool = ctx.enter_context(tc.tile_pool(name="weights", bufs=1))
            xpool = ctx.enter_context(tc.tile_pool(name="xstream", bufs=8))
            qkpool = ctx.enter_context(tc.tile_pool(name="qk", bufs=1))
            vpool = ctx.enter_context(tc.tile_pool(name="v", bufs=1))
            otpool = ctx.enter_context(tc.tile_pool(name="ot", bufs=1))
            ppool = ctx.enter_context(tc.tile_pool(name="p", bufs=4))
            rcpool = ctx.enter_context(tc.tile_pool(name="rc", bufs=4))
            bcpool = ctx.enter_context(tc.tile_pool(name="bc", bufs=4))
            opool = ctx.enter_context(tc.tile_pool(name="oev", bufs=4))
            # PSUM (8 banks): "s" 2x2 banks (score tiles), "ot" 2x1 bank
            # (AV accumulators, live per attn_pair), "acc" 2x1 bank
            # (rotating projection / Wo accumulators).
            ps_p = ctx.enter_context(tc.tile_pool(name="ps_p", bufs=2, space="PSUM"))
            ps_s = ctx.enter_context(tc.tile_pool(name="ps_s", bufs=2, space="PSUM"))

            # ---- constants / weights ----
            wq_s = wpool.tile([P, NKC, GP], bf16, tag="wq")
            wk_s = wpool.tile([P, NKC, GP], bf16, tag="wk")
            wv_s = wpool.tile([P, NKC, GP], bf16, tag="wv")
            wo_s = wpool.tile([P, GP // P, D], bf16, tag="wo")
            mask_s = wpool.tile([P, NKT_MAX], bf16, tag="mask")
            nc.sync.dma_start(wk_s[:], wk.rearrange("(c p) m -> p c m", p=P))
            nc.sync.dma_start(wq_s[:], wq.rearrange("(c p) m -> p c m", p=P))
            nc.sync.dma_start(wv_s[:], wv.rearrange("(c p) m -> p c m", p=P))
            nc.sync.dma_start(mask_s[:], maskd[:])
            nc.sync.dma_start(wo_s[:], wo.rearrange("(c p) m -> p c m", p=P))

            xq_r = xq.rearrange("(c p) s -> p c s", p=P)
            xk_r = xk.rearrange("(c p) s -> p c s", p=P)
            xv_r = xv.rearrange("(c p) s -> p c s", p=P)

            from contextlib import nullcontext
            for _ in range(reps):
              with (tc.For_i(0, loop_n, 1) if loop_n > 1 else nullcontext()):
                  # ---- residents ----
                  QT = [qkpool.tile([P, S], bf16, tag=f"qt{t}", name=f"qt{t}") for t in range(2)]
                  KT = [qkpool.tile([P, S], bf16, tag=f"kt{t}", name=f"kt{t}") for t in range(2)]
                  V_sb = vpool.tile([P, NKT_MAX, HG, DH + 1], bf16, tag="vsb")
                  OT = [otpool.tile([P, S], bf16, tag=f"ot{t}", name=f"ot{t}") for t in range(2)]

                  # mask columns of V (denominator trick), all key tiles at once
                  for h in range(HG):
                      nc.gpsimd.tensor_copy(
                          V_sb[:, 0:nkt, h, DH : DH + 1], mask_s[:, 0:nkt]
                      )

                  # (deadline, closure) units woven into the kt loops.
                  # A unit with deadline D is guaranteed to be emitted by the
                  # top of attention iteration D-1 (before that iteration's
                  # lookahead scores), regardless of queue position.
                  fillers = []
                  FAR = 10**9

                  def x_dma(x_r, qb, cols=QB):
                      xt = xpool.tile([P, NKC, QB], bf16, tag="xs", name="xt")
                      nc.sync.dma_start(
                          xt[:, :, 0:cols], x_r[:, :, qb * QB : qb * QB + cols]
                      )
                      return xt

                  def qk_proj_half(w_t, dst, qb, xt, crange, cols=QB):
                      """Half of a projection block: contraction chunks crange,
                      both output tiles, as row-group-crossed K=64 pairs."""
                      if crange[0] == 0:
                          ps = [ps_p.tile([P, QB], f32, tag="acc", name=f"pj{m}",
                                          bufs=2) for m in range(2)]
                          qk_proj_half.ps = ps
                      else:
                          ps = qk_proj_half.ps
                      last = NKC - 1
                      for c in crange:
                          en = c == last
                          if SPLIT_PROJ:
                              nc.tensor.matmul(ps[0][:, 0:cols], lhsT=w_t[LO, c, 0:P],
                                               rhs=xt[LO, c, 0:cols],
                                               start=(c == 0), stop=False)
                              nc.tensor.matmul(ps[1][:, 0:cols], lhsT=w_t[HI, c, P:GP],
                                               rhs=xt[HI, c, 0:cols],
                                               start=(c == 0), stop=False)
                              nc.tensor.matmul(ps[1][:, 0:cols], lhsT=w_t[LO, c, P:GP],
                                               rhs=xt[LO, c, 0:cols],
                                               start=False, stop=en)
                              nc.tensor.matmul(ps[0][:, 0:cols], lhsT=w_t[HI, c, 0:P],
                                               rhs=xt[HI, c, 0:cols],
                                               start=False, stop=en)
                          else:
                              for mt in range(2):
                                  nc.tensor.matmul(
                                      ps[mt][:, 0:cols],
                                      lhsT=w_t[:, c, mt * P : (mt + 1) * P],
                                      rhs=xt[:, c, 0:cols],
                                      start=(c == 0), stop=en)
                      if crange[-1] == last:
                          for mt in range(2):
                              nc.vector.tensor_copy(
                                  dst[mt][:, qb * QB : qb * QB + cols],
                                  ps[mt][:, 0:cols],
                              )

                  def push_qk_proj(x_r, w_t, dst, qb, cols=QB, deadline=FAR):
                      xt = x_dma(x_r, qb, cols)
                      fillers.append((deadline,
                                      lambda: qk_proj_half(w_t, dst, qb, xt,
                                                           range(0, 4), cols)))
                      fillers.append((deadline,
                                      lambda: qk_proj_half(w_t, dst, qb, xt,
                                                           range(4, NKC), cols)))

                  def v_proj_pair(xt, qb, i0, two):
                      """V projection for st pair (i0, i0+1) as crossed K=64
                      pairs, or a single serial st when two=False."""
                      st0 = qb * (QB // P) + i0
                      if two:
                          ps = [ps_p.tile([P, GP], f32, tag="acc", name=f"vp{j}",
                                          bufs=2) for j in range(2)]
                          for c in range(NKC):
                              st_, en = (c == 0), (c == NKC - 1)
                              nc.tensor.matmul(ps[0][:], lhsT=xt[LO, c, i0 * P:(i0 + 1) * P],
                                               rhs=wv_s[LO, c, :], start=st_, stop=False)
                              nc.tensor.matmul(ps[1][:], lhsT=xt[HI, c, (i0 + 1) * P:(i0 + 2) * P],
                                               rhs=wv_s[HI, c, :], start=st_, stop=False)
                              nc.tensor.matmul(ps[1][:], lhsT=xt[LO, c, (i0 + 1) * P:(i0 + 2) * P],
                                               rhs=wv_s[LO, c, :], start=False, stop=en)
                              nc.tensor.matmul(ps[0][:], lhsT=xt[HI, c, i0 * P:(i0 + 1) * P],
                                               rhs=wv_s[HI, c, :], start=False, stop=en)
                          for j in range(2):
                              nc.vector.tensor_copy(
                                  V_sb[:, st0 + j, :, 0:DH],
                                  ps[j][:].rearrange("p (h d) -> p h d", h=HG),
                              )
                      else:
                          ps = ps_p.tile([P, GP], f32, tag="acc", name="vp0", bufs=2)
                          for c in range(NKC):
                              nc.tensor.matmul(ps[:], lhsT=xt[:, c, i0 * P:(i0 + 1) * P],
                                               rhs=wv_s[:, c, :],
                                               start=(c == 0), stop=(c == NKC - 1))
                          nc.vector.tensor_copy(
                              V_sb[:, st0, :, 0:DH],
                              ps[:].rearrange("p (h d) -> p h d", h=HG),
                          )

                  def push_v_proj(qb, nst):
                      xt = x_dma(xv_r, qb, nst * P)
                      i = 0
                      while i < nst:
                          two = SPLIT_PROJ and i + 1 < nst
                          st = qb * (QB // P) + i
                          fillers.append((min(st + 2, nkt),
                                          lambda x=xt, q=qb, i0=i, t=two:
                                          v_proj_pair(x, q, i0, t)))
                          i += 2 if two else 1

                  # ---- attention: one flat pipeline over (q-block, head
                  # pair, key tile) so the Act engine's exp stream never
                  # breaks at pair/q-block boundaries (scores lookahead
                  # crosses them).
                  def scores(pr, qb, kt):
                      qsl = slice(qb * QB, (qb + 1) * QB)
                      s_ps = ps_s.tile([P, 2, QB], f32, tag="s", name="sps")
                      for hp in range(2):
                          hsl = slice(hp * DH, (hp + 1) * DH)
                          nc.tensor.matmul(
                              s_ps[:, hp, :],
                              lhsT=KT[pr][hsl, kt * P : (kt + 1) * P],
                              rhs=QT[pr][hsl, qsl],
                              start=True,
                              stop=True,
                          )
                      return s_ps

                  def av(ot_ps, pr, pt, kt, en):
                      st_ = kt == 0
                      for hp in range(2):
                          nc.tensor.matmul(
                              ot_ps[hp][:],
                              lhsT=V_sb[:, kt, 2 * pr + hp, :],
                              rhs=pt[:, hp, :], start=st_, stop=en)

                  def normalize(ot_ps, pr, qb):
                      # reciprocal of denominators (psum row 64), move to
                      # partition 0 by DMA, broadcast on gpsimd, scale.
                      qsl = slice(qb * QB, (qb + 1) * QB)
                      for hp in range(2):
                          rc = rcpool.tile([DH + 1, QB], bf16, tag="rc", name="rc")
                          with nc.allow_low_precision(reason="bf16 normalization"):
                              nc.vector.reciprocal(
                                  rc[DH : DH + 1, :], ot_ps[hp][DH : DH + 1, :]
                              )
                          rc0 = rcpool.tile([1, QB], bf16, tag="rc0", name="rc0")
                          nc.sync.dma_start(rc0[0:1, :], rc[DH : DH + 1, :])
                          bc = bcpool.tile([P, QB], bf16, tag="bc", name="bc")
                          nc.gpsimd.partition_broadcast(bc[:], rc0[0:1, :])
                          with nc.allow_low_precision(reason="bf16 normalization"):
                              if hp == 0:
                                  nc.vector.tensor_copy(
                                      OT[pr][0:DH, qsl], ot_ps[hp][0:DH, :]
                                  )
                                  nc.vector.tensor_mul(
                                      OT[pr][0:DH, qsl], OT[pr][0:DH, qsl], bc[0:DH, :]
                                  )
                              else:
                                  oev = opool.tile([DH, QB], bf16, tag="oev", name="oev")
                                  nc.vector.tensor_copy(oev[:], ot_ps[hp][0:DH, :])
                                  nc.sync.dma_start(OT[pr][DH:P, qsl], oev[:])
                                  nc.vector.tensor_mul(
                                      OT[pr][DH:P, qsl], OT[pr][DH:P, qsl], bc[DH:P, :]
                                  )

                  def attention(units, unit_start_cb=None):
                      """units: list of (qb, pr). Flat software pipeline;
                      filler deadlines are in global iteration numbers."""
                      n = len(units)
                      s_cur = scores(units[0][1], units[0][0], 0)
                      for ui, (qb, pr) in enumerate(units):
                          if unit_start_cb is not None:
                              unit_start_cb(ui)
                          ot_ps = [
                              ps_p.tile([DH + 1, QB], f32, tag="ot",
                                        name=f"otps{i}", bufs=2)
                              for i in range(2)
                          ]
                          prev = None
                          for kt in range(nkt):
                              g = ui * nkt + kt
                              popped = False
                              while fillers and (
                                  not popped
                                  or min(dl for dl, _ in fillers) <= g + 1
                              ):
                                  fillers.pop(0)[1]()
                                  popped = True
                              if kt + 1 < nkt:
                                  s_next = scores(pr, qb, kt + 1)
                              elif ui + 1 < n:
                                  s_next = scores(units[ui + 1][1],
                                                  units[ui + 1][0], 0)
                              else:
                                  s_next = None
                              pt = ppool.tile([P, 2, QB], bf16, tag="p", name="pt")
                              nc.scalar.activation(
                                  pt[:], s_cur[:],
                                  bass.mybir.ActivationFunctionType.Exp
                              )
                              s_cur = s_next
                              if prev is not None:
                                  av(ot_ps, pr, prev, kt - 1, False)
                              prev = pt
                          av(ot_ps, pr, prev, nkt - 1, True)
                          normalize(ot_ps, pr, qb)

                  def wo_st(st):
                      ssl = slice(st * P, (st + 1) * P)
                      ps = [ps_p.tile([P, QB], f32, tag="acc", name=f"wo{n}", bufs=2)
                            for n in range(2)]
                      for t in range(2):
                          st_, en = (t == 0), (t == 1)
                          if SPLIT_PROJ:
                              nc.tensor.matmul(ps[0][:], lhsT=OT[t][LO, ssl],
                                               rhs=wo_s[LO, t, 0:QB], start=st_, stop=False)
                              nc.tensor.matmul(ps[1][:], lhsT=OT[t][HI, ssl],
                                               rhs=wo_s[HI, t, QB:D], start=st_, stop=False)
                              nc.tensor.matmul(ps[1][:], lhsT=OT[t][LO, ssl],
                                               rhs=wo_s[LO, t, QB:D], start=False, stop=en)
                              nc.tensor.matmul(ps[0][:], lhsT=OT[t][HI, ssl],
                                               rhs=wo_s[HI, t, 0:QB], start=False, stop=en)
                          else:
                              for nh in range(2):
                                  nc.tensor.matmul(
                                      ps[nh][:], lhsT=OT[t][:, ssl],
                                      rhs=wo_s[:, t, nh * QB : (nh + 1) * QB],
                                      start=st_, stop=en)
                      for nh in range(2):
                          osb = opool.tile([P, QB], f32, tag="osb", name="osb")
                          nc.vector.tensor_copy(osb[:], ps[nh][:])
                          nc.sync.dma_start(outp[ssl, nh * QB : (nh + 1) * QB], osb[:])

                  def kv_block_cols(b):
                      return min(QB, kv_cols - b * QB)

                  # ---- emission ----
                  # startup: block 0 of K/Q/V computed inline, then the
                  # remaining K/V blocks woven one filler unit per kt of the
                  # first attention pass.
                  xt_k0 = x_dma(xk_r, 0, kv_block_cols(0))
                  xt_q0 = x_dma(xq_r, 0)
                  qk_proj_half(wk_s, KT, 0, xt_k0, range(0, 4), kv_block_cols(0))
                  qk_proj_half(wk_s, KT, 0, xt_k0, range(4, NKC), kv_block_cols(0))
                  qk_proj_half(wq_s, QT, 0, xt_q0, range(0, 4))
                  qk_proj_half(wq_s, QT, 0, xt_q0, range(4, NKC))
                  push_v_proj(0, min(QB // P, nkt))
                  for b in range(1, nkb):
                      cols = kv_block_cols(b)
                      push_qk_proj(xk_r, wk_s, KT, b, cols, deadline=4 * b)
                      push_v_proj(b, (cols + P - 1) // P)

                  units = [(qb, pr) for qb in range(NQB) for pr in range(2)]

                  def unit_start_cb(ui):
                      qb, pr = units[ui]
                      if pr != 0:
                          return
                      # entering q-block qb: queue Wo of qb-1 and the Q
                      # projection of qb+1. Q proj must land before the
                      # cross-unit lookahead scores of (qb+1, 0), one
                      # iteration before that unit starts.
                      if qb >= 1:
                          for st in range((qb - 1) * (QB // P), qb * (QB // P)):
                              fillers.append((FAR, lambda s=st: wo_st(s)))
                      if qb + 1 < NQB:
                          push_qk_proj(xq_r, wq_s, QT, qb + 1,
                                       deadline=(2 * qb + 2) * nkt)

                  attention(units, unit_start_cb)
                  while fillers:
                      fillers.pop(0)[1]()
                  for st in range((NQB - 1) * (QB // P), NQB * (QB // P)):
                      wo_st(st)

    nc.compile()
    _BUILD_CACHE[key] = nc
    return nc


def _prep_inputs(queries, keys, values, Wq, Wk, Wv, Wo, valid_seq_lens):
    import ml_dtypes

    bf = ml_dtypes.bfloat16
    qn = np.asarray(queries, dtype=np.float32)
    kn = np.asarray(keys, dtype=np.float32)
    vn = np.asarray(values, dtype=np.float32)
    wqn = (np.asarray(Wq, dtype=np.float32) * np.float32(1.0 / np.sqrt(DH))).astype(bf)
    wkn = np.asarray(Wk, dtype=np.float32).astype(bf)
    wvn = np.asarray(Wv, dtype=np.float32).astype(bf)
    won = np.asarray(Wo, dtype=np.float32).astype(bf)
    vl = np.asarray(valid_seq_lens).astype(np.int64)

    in_maps = []
    for core in range(8):
        b, g = core // 4, core % 4
        gsl = slice(g * GP, (g + 1) * GP)
        vmask = (np.arange(S) < vl[b]).astype(np.float32)
        vb = vn[b] * vmask[:, None]  # zero masked value rows
        in_maps.append(
            {
                "xq_t": np.ascontiguousarray(qn[b].T.astype(bf)),
                "xk_t": np.ascontiguousarray(kn[b].T.astype(bf)),
                "xv_t": np.ascontiguousarray(vb.T.astype(bf)),
                "wq": np.ascontiguousarray(wqn[:, gsl]),
                "wk": np.ascontiguousarray(wkn[:, gsl]),
                "wv": np.ascontiguousarray(wvn[:, gsl]),
                "wo": np.ascontiguousarray(won[gsl, :]),
                "mask_t": np.ascontiguousarray(
                    vmask.reshape(NKT_MAX, P).T.astype(bf)
                ),
            }
        )
    return in_maps


def kernel(queries, keys, values, Wq, Wk, Wv, Wo, valid_seq_lens):
    from concourse.bass_utils import run_bass_kernel_spmd

    nc = _build(nkt=nkt_for(valid_seq_lens))
    in_maps = _prep_inputs(
        queries, keys, values, Wq, Wk, Wv, Wo, valid_seq_lens
    )
    res = run_bass_kernel_spmd(nc, in_maps, list(range(8)))
    out = np.zeros((B, S, D), dtype=np.float32)
    for core in range(8):
        out[core // 4] += res.results[core]["out_p"]
    return out
